# revision 1
# baseline (speedup 1.0000x reference)
"""Trainium2 Bass kernel for the CriticalField PDE step (fp16/int8 pipeline).

Computes one explicit step of a coupled magnitude/phase field update on a
4096x4096 grid with circular boundary conditions:

    mag_lap   = 4-neighbor circular Laplacian of magnitude
    phase_lap = 4-neighbor circular Laplacian of phase
    d_mag     = tension*mag_lap - damping*mag - nonlinearity*mag^3
    d_phase   = tension*phase_lap + COUPLING*sin(up(phase) - phase)
    out[0]    = clip(mag + DT*d_mag, -2, 2)
    out[1]    = clip(phase + DT*d_phase, 0, 2*pi)

HBM bandwidth is the measured bottleneck (all 8 cores share the HBM path;
compute hides entirely under the DMA), so the kernel minimizes bytes moved.
The rel-err budget (2e-2) is loose enough for narrow dtypes:
  magnitude in:  fp16 (2B) - kept wider because the mag^3 term amplifies
                 input quantization ~3*nonlin*DT*mag^2 ~ 1.6x at mag=6
  phase in:      uint8 (1B), uniform over [0, 2pi]  (quant err 1.2e-2 abs,
                 but only ~0.7x of it reaches the output)
  mag out:       int8, scale 63.5  (exact clip range [-2,2] -> [-127,127])
  phase out:     uint8, scale 255/2pi (exact clip range [0,2pi] -> [0,255])
Host converts/quantizes inputs and dequantizes outputs (free for the device).
Total HBM traffic: 5 bytes/element vs 8 (all-fp32) baseline's 32.

Sharding: rows split across 8 NeuronCores; each core gets 504 rows as 4
tiles of 128 partitions (126 valid rows each) plus 1/8 of the 64 leftover
rows as a column-split overflow block. Row halos are materialized host-side.
Column halos for main tiles are produced on-device by copying the wrap
columns inside each loaded tile (keeps every DMA line aligned).

Per-core compute (fp16 data, f32 PSUM accumulation), per 512-col block:
  ScalarE: ph16 = Copy((2pi/255) * ph_u8)  [dequant, once per tile]
           c2 = Square(sqrt(SM*Cc)*mg_c);  s = Sin(pa)
  TensorE: pm = SM*[(B*ud + A*I)@mg_c + B*I@(mg_l + mg_r)]         (3 MM)
           pa = (up - center)@ph16_c                                (1 MM)
           pp = SP*[(B*ud + A2*I)@ph16_c + B*I@(ph_l+ph_r) + K*I@s] (4 MM)
  VectorE: c3 = c2*mg_c; mm = pm - c3; out_mag = clip(mm, +-127) -> int8
           out_phase = clip(pp, 0, 255) -> uint8
All scale factors (including the output quantization scales SM/SP) live in
the fp16 weights, so PSUM holds the finished scaled update and each field
needs only one PSUM-drain op.
"""

import numpy as np

SIZE = 4096
NCORES = 8
TILE_VALID = 126
NTILES = 4
MAIN_ROWS = TILE_VALID * NTILES          # 504 rows per core via main tiles
OVF_ROWS = SIZE - MAIN_ROWS * NCORES     # 64 leftover rows (4032..4095)
OVF_COLS = SIZE // NCORES                # 512 columns of overflow per core
DT = 0.05
COUPLING = 0.015
TWO_PI = 2.0 * np.pi
SM = 63.5                                # mag output quant scale
SP = 255.0 / TWO_PI                      # phase quant scale (in and out)

_PROG_CACHE: dict = {}
_WEIGHTS_CACHE: dict = {}


def _banded_weights(damping, tension):
    """lhsT weight matrices for nc.tensor.matmul (out = lhsT.T @ rhs).

    lhsT[k, m] = contribution of rhs partition k to output partition m.
    Output partition m corresponds to slab row t+m; its row-neighbors are
    tile partitions m-1 (up) and m+1 (down). Scale factors are folded in:
    B on the stencil bands, A/A2 on the center diagonals, K on the sin
    injection, and the output quant scales SM/SP on the whole path, so
    PSUM accumulates the finished pre-clip scaled update.
    """
    key = (float(damping), float(tension))
    if key in _WEIGHTS_CACHE:
        return _WEIGHTS_CACHE[key]
    A = 1.0 - 4.0 * DT * tension - DT * damping
    A2 = 1.0 - 4.0 * DT * tension
    B = DT * tension
    K = DT * COUPLING
    idx = np.arange(127)
    w_ud = np.zeros((128, 128), np.float32)
    w_ud[idx, idx + 1] = 1.0      # k = m-1 -> up neighbor
    w_ud[idx + 1, idx] = 1.0      # k = m+1 -> down neighbor
    eye = np.eye(128, dtype=np.float32)
    w_m_c = SM * (B * w_ud + A * eye)
    w_p_c = SP * (B * w_ud + A2 * eye)
    w_eyeBm = SM * B * eye
    w_eyeBp = SP * B * eye
    w_umi = np.zeros((128, 128), np.float32)
    w_umi[idx, idx + 1] = 1.0     # +up
    w_umi -= eye                  # -center
    w_sinK = SP * K * eye
    w_all = np.concatenate(
        [w_m_c, w_p_c, w_eyeBm, w_eyeBp, w_umi, w_sinK],
        axis=1).astype(np.float16)
    _WEIGHTS_CACHE[key] = {"w_all": np.ascontiguousarray(w_all)}
    return _WEIGHTS_CACHE[key]


def _build_program(Cc, repeat=1, mode="full", hw_loop=False):
    import concourse.bass as bass
    import concourse.bacc as bacc
    import concourse.tile as tile
    from concourse import mybir

    f16 = mybir.dt.float16
    f32 = mybir.dt.float32
    u8 = mybir.dt.uint8
    i8 = mybir.dt.int8
    Act = mybir.ActivationFunctionType
    Alu = mybir.AluOpType

    nc = bacc.Bacc(trn_type="TRN2", target_bir_lowering=False, debug=False)

    mag_slab = nc.dram_tensor("mag_slab", [MAIN_ROWS + 2, SIZE], f16,
                              kind="ExternalInput").ap()
    ph_slab = nc.dram_tensor("ph_slab", [MAIN_ROWS + 2, SIZE], u8,
                             kind="ExternalInput").ap()
    mag_ovf = nc.dram_tensor("mag_ovf", [OVF_ROWS + 2, OVF_COLS + 2], f16,
                             kind="ExternalInput").ap()
    ph_ovf = nc.dram_tensor("ph_ovf", [OVF_ROWS + 2, OVF_COLS + 2], u8,
                            kind="ExternalInput").ap()
    w_all_d = nc.dram_tensor("w_all", [128, 768], f16, kind="ExternalInput").ap()
    out_mag = nc.dram_tensor("out_mag", [MAIN_ROWS, SIZE], i8,
                             kind="ExternalOutput").ap()
    out_ph = nc.dram_tensor("out_ph", [MAIN_ROWS, SIZE], u8,
                            kind="ExternalOutput").ap()
    out_ovf_mag = nc.dram_tensor("out_ovf_mag", [OVF_ROWS, OVF_COLS], i8,
                                 kind="ExternalOutput").ap()
    out_ovf_ph = nc.dram_tensor("out_ovf_ph", [OVF_ROWS, OVF_COLS], u8,
                                kind="ExternalOutput").ap()

    sCc = float(np.sqrt(SM * Cc))
    DQ = float(TWO_PI / 255.0)

    with tile.TileContext(nc) as tc:
        with (
            tc.tile_pool(name="wts", bufs=1) as wpool,
            tc.tile_pool(name="inp", bufs=3) as inp,
            tc.tile_pool(name="phd", bufs=3) as phd,
            tc.tile_pool(name="outp", bufs=2) as outp,
            tc.tile_pool(name="sml", bufs=4) as sml,
            tc.tile_pool(name="psm", bufs=3, space="PSUM") as psm,
            tc.tile_pool(name="psp", bufs=3, space="PSUM") as psp,
            tc.tile_pool(name="psb", bufs=2, space="PSUM") as psb,
        ):
            w_all = wpool.tile([128, 768], f16, tag="w_all")
            nc.sync.dma_start(w_all[:, :], w_all_d[:, :])

            def emit_block(mg, ph, om, op_, P, ncols):
                """Compute for one loaded tile.

                mg/ph: fp16 input tiles [P, ncols+2] (col halo at both ends)
                om/op_: output tiles [P, ncols] (i8/u8); valid parts 1..P-2.
                mode ladder (timing diagnostics): "dma" = loads/stores only;
                "pe" = +matmuls; "peact" = +ScalarE ops; "full" = everything.
                """
                if mode == "dma":
                    nc.vector.tensor_copy(om[0:P, 0:ncols], mg[0:P, 1:1 + ncols])
                    nc.vector.tensor_copy(op_[0:P, 0:ncols], ph[0:P, 1:1 + ncols])
                    return
                do_act = mode in ("peact", "full")
                do_rest = mode == "full"
                w_m_c = w_all[0:P, 0:P]
                w_p_c = w_all[0:P, 128:128 + P]
                w_eyeBm = w_all[0:P, 256:256 + P]
                w_eyeBp = w_all[0:P, 384:384 + P]
                w_umi = w_all[0:P, 512:512 + P]
                w_sinK = w_all[0:P, 640:640 + P]
                for j in range(0, ncols, 512):
                    cw = min(512, ncols - j)
                    mg_c = mg[0:P, 1 + j:1 + j + cw]
                    mg_l = mg[0:P, j:j + cw]
                    mg_r = mg[0:P, 2 + j:2 + j + cw]
                    ph_c = ph[0:P, 1 + j:1 + j + cw]
                    ph_l = ph[0:P, j:j + cw]
                    ph_r = ph[0:P, 2 + j:2 + j + cw]

                    pa = psb.tile([P, cw], f32, tag="pa")
                    nc.tensor.matmul(pa[:, :], w_umi, ph_c, start=True, stop=True)
                    if do_act:
                        s = sml.tile([P, cw], f16, tag="s")
                        nc.scalar.activation(s[:, :], pa[:, :], Act.Sin)
                        c2 = sml.tile([P, cw], f16, tag="c2")
                        nc.scalar.activation(c2[:, :], mg_c, Act.Square,
                                             bias=0.0, scale=sCc)
                    pm = psm.tile([P, cw], f32, tag="pm")
                    nc.tensor.matmul(pm[:, :], w_m_c, mg_c, start=True, stop=False)
                    nc.tensor.matmul(pm[:, :], w_eyeBm, mg_l, start=False, stop=False)
                    nc.tensor.matmul(pm[:, :], w_eyeBm, mg_r, start=False, stop=True)
                    pp = psp.tile([P, cw], f32, tag="pp")
                    nc.tensor.matmul(pp[:, :], w_p_c, ph_c, start=True, stop=False)
                    nc.tensor.matmul(pp[:, :], w_eyeBp, ph_l, start=False, stop=False)
                    if do_act:
                        nc.tensor.matmul(pp[:, :], w_eyeBp, ph_r,
                                         start=False, stop=False)
                        nc.tensor.matmul(pp[:, :], w_sinK, s[:, :],
                                         start=False, stop=True)
                    else:
                        nc.tensor.matmul(pp[:, :], w_eyeBp, ph_r,
                                         start=False, stop=True)
                    if not do_rest:
                        continue
                    c3 = sml.tile([P, cw], f16, tag="c3")
                    nc.vector.tensor_tensor(c3[:, :], c2[:, :], mg_c, Alu.mult)
                    mm = sml.tile([P, cw], f16, tag="mm")
                    nc.vector.tensor_tensor(mm[:, :], pm[:, :], c3[:, :],
                                            Alu.subtract)
                    nc.vector.tensor_scalar(
                        om[0:P, j:j + cw], mm[:, :],
                        127.0, -127.0, Alu.min, Alu.max)
                    nc.vector.tensor_scalar(
                        op_[0:P, j:j + cw], pp[:, :],
                        0.0, 255.0, Alu.max, Alu.min)
                if mode in ("pe", "peact"):
                    nc.vector.tensor_copy(om[0:P, 0:ncols], mg[0:P, 1:1 + ncols])
                    nc.vector.tensor_copy(op_[0:P, 0:ncols], ph[0:P, 1:1 + ncols])

            HALF = SIZE // 2

            def emit_rep():
              # Overflow block first: its small ops fill the pipeline-fill
              # bubble while the first big tile's DMA is still in flight.
              P = OVF_ROWS + 2
              mg = inp.tile([P, OVF_COLS + 2], f16, tag="mgo")
              nc.sync.dma_start(mg[:, :], mag_ovf[:, :])
              q8 = inp.tile([P, OVF_COLS + 2], u8, tag="qo")
              nc.sync.dma_start(q8[:, :], ph_ovf[:, :])
              ph = phd.tile([P, OVF_COLS + 2], f16, tag="pho")
              nc.scalar.activation(ph[:, :], q8[:, :], Act.Copy,
                                   bias=0.0, scale=DQ)
              om = outp.tile([P, OVF_COLS], i8, tag="omo")
              op_ = outp.tile([P, OVF_COLS], u8, tag="opo")
              emit_block(mg, ph, om, op_, P, OVF_COLS)
              nc.sync.dma_start(out_ovf_mag[:, :], om[1:P - 1, :])
              nc.sync.dma_start(out_ovf_ph[:, :], op_[1:P - 1, :])

              def load_tile(ti):
                t0 = TILE_VALID * ti
                mg = inp.tile([128, SIZE + 2], f16, tag="mg")
                nc.sync.dma_start(mg[:, 1:1 + SIZE], mag_slab[t0:t0 + 128, :])
                q8 = inp.tile([128, SIZE], u8, tag="q8")
                nc.sync.dma_start(q8[:, :], ph_slab[t0:t0 + 128, :])
                return mg, q8

              def prep_tile(mg, q8):
                # Circular column halos: col 0 <- data col 4095, col 4097 <-
                # data col 0 (both already present inside the loaded tile).
                nc.vector.tensor_copy(mg[:, 0:1], mg[:, SIZE:SIZE + 1])
                nc.vector.tensor_copy(mg[:, SIZE + 1:SIZE + 2], mg[:, 1:2])
                ph = phd.tile([128, SIZE + 2], f16, tag="ph")
                nc.scalar.activation(ph[:, 1:1 + SIZE], q8[:, :], Act.Copy,
                                     bias=0.0, scale=DQ)
                nc.vector.tensor_copy(ph[:, 0:1], ph[:, SIZE:SIZE + 1])
                nc.vector.tensor_copy(ph[:, SIZE + 1:SIZE + 2], ph[:, 1:2])
                return ph

              cur = load_tile(0)
              cur_ph = prep_tile(*cur)
              for ti in range(NTILES):
                if ti + 1 < NTILES:
                    nxt = load_tile(ti + 1)
                t0 = TILE_VALID * ti
                om = outp.tile([128, SIZE], i8, tag="om")
                op_ = outp.tile([128, SIZE], u8, tag="op")
                emit_block(cur[0], cur_ph, om, op_, 128, SIZE)
                # Drain each output in column halves so the store of the
                # first half overlaps the clips of the second.
                for lo in (0, HALF):
                    nc.sync.dma_start(
                        out_mag[t0:t0 + TILE_VALID, lo:lo + HALF],
                        om[1:127, lo:lo + HALF])
                    nc.sync.dma_start(
                        out_ph[t0:t0 + TILE_VALID, lo:lo + HALF],
                        op_[1:127, lo:lo + HALF])
                if ti + 1 < NTILES:
                    cur = nxt
                    cur_ph = prep_tile(*cur)

            if hw_loop and repeat > 1:
                with tc.For_i(0, repeat, 1):
                    emit_rep()
            else:
                for _rep in range(repeat):
                    emit_rep()

    nc.compile()
    return nc


def _get_program(damping, tension, nonlinearity, repeat=1, mode="full",
                 hw_loop=False):
    key = (damping, tension, nonlinearity, repeat, mode, hw_loop)
    if key not in _PROG_CACHE:
        Cc = DT * nonlinearity
        _PROG_CACHE[key] = _build_program(Cc, repeat, mode, hw_loop)
    return _PROG_CACHE[key]


def _make_in_maps(mag, ph, damping=0.05, tension=1.5):
    """Per-core input dicts: fp16 mag, uint8 phase, circular row halos."""
    w = _banded_weights(damping, tension)
    mag16 = mag.astype(np.float16)
    ph8 = np.clip(np.rint(ph * SP), 0, 255).astype(np.uint8)
    cols = np.arange(-1, SIZE + 1) % SIZE
    ovf_rows = np.arange(MAIN_ROWS * NCORES - 1, SIZE + 1) % SIZE
    mag_ovf_full = mag16[np.ix_(ovf_rows, cols)]
    ph_ovf_full = ph8[np.ix_(ovf_rows, cols)]
    in_maps = []
    for m in range(NCORES):
        rows = np.arange(MAIN_ROWS * m - 1, MAIN_ROWS * (m + 1) + 1) % SIZE
        c0 = OVF_COLS * m
        in_maps.append({
            "mag_slab": np.ascontiguousarray(mag16[rows, :]),
            "ph_slab": np.ascontiguousarray(ph8[rows, :]),
            "mag_ovf": np.ascontiguousarray(mag_ovf_full[:, c0:c0 + OVF_COLS + 2]),
            "ph_ovf": np.ascontiguousarray(ph_ovf_full[:, c0:c0 + OVF_COLS + 2]),
            "w_all": w["w_all"],
        })
    return in_maps


def _assemble(results):
    out = np.empty((1, 2, SIZE, SIZE), np.float32)
    for m in range(NCORES):
        r = results[m]
        r0, r1 = MAIN_ROWS * m, MAIN_ROWS * (m + 1)
        out[0, 0, r0:r1, :] = r["out_mag"].astype(np.float32) / SM
        out[0, 1, r0:r1, :] = r["out_ph"].astype(np.float32) * (TWO_PI / 255.0)
        c0, c1 = OVF_COLS * m, OVF_COLS * (m + 1)
        out[0, 0, MAIN_ROWS * NCORES:, c0:c1] = \
            r["out_ovf_mag"].astype(np.float32) / SM
        out[0, 1, MAIN_ROWS * NCORES:, c0:c1] = \
            r["out_ovf_ph"].astype(np.float32) * (TWO_PI / 255.0)
    return out


def kernel(magnitude, phase, damping, tension, nonlinearity):
    from concourse.bass_utils import run_bass_kernel_spmd

    mag = np.asarray(magnitude, dtype=np.float32).reshape(SIZE, SIZE)
    ph = np.asarray(phase, dtype=np.float32).reshape(SIZE, SIZE)
    d = float(np.asarray(damping))
    tn = float(np.asarray(tension))
    nl = float(np.asarray(nonlinearity))

    nc = _get_program(d, tn, nl)
    in_maps = _make_in_maps(mag, ph, d, tn)
    res = run_bass_kernel_spmd(nc, in_maps, core_ids=list(range(NCORES)))
    return _assemble(res.results)

